# revision 3
# baseline (speedup 1.0000x reference)
"""GCN layer kernel v4 for Trainium2 (8 NeuronCores).

v2 + pipeline fixes:
  - balanced quarter split (25000 rows) + robust packer -> csum=16
  - SW=128 one-hots; pad slots -> 999 (all-zero row)
  - slotd preloaded once (split into lo/hi chunk halves)
  - is_equal split across DVE and GpSimd per group (parallel build)
  - DMA issue spread across sequencers (idx: sync, xod: scalar, out: vector)
  - deeper ework buffering
"""
import sys
sys.path.insert(0, "/opt/trn_rl_repo")

import numpy as np
import ml_dtypes

import concourse.bass as bass
import concourse.mybir as mybir
import concourse.tile as tile
from concourse import bacc, bass_utils

F32 = mybir.dt.float32
BF16 = mybir.dt.bfloat16
I16 = mybir.dt.int16

N_NODES = 100000
D = 128
NCORES = 8
SW = 128
PAD_SLOT = 999
GATHER_CAP = 1024

_BUILD_CACHE = {}
LAST_RESULTS = None


def _plan_groups(bpc, gmax):
    groups = []
    left = bpc
    while left > 0:
        g = min(gmax, left)
        groups.append(g)
        left -= g
    return groups


def _build(c_list, groups, bpc, npad, qrows, cap):
    key = (tuple(c_list), tuple(groups), bpc, npad, qrows, cap)
    if key in _BUILD_CACHE:
        return _BUILD_CACHE[key]

    csum = int(sum(c_list))
    half = csum // 2
    totslots = bpc * csum * 128
    npc = bpc * 128

    nc = bacc.Bacc("TRN2", target_bir_lowering=False, debug=False,
                   num_devices=NCORES, num_swdge_queues=4)
    xq = nc.dram_tensor("xq", [npad, D], BF16, kind="ExternalInput")
    idxd = nc.dram_tensor("idxd", [128, totslots // 16], I16,
                          kind="ExternalInput")
    slotv = nc.dram_tensor("slotv", [128, bpc * half], BF16,
                           kind="ExternalInput")
    slotp = nc.dram_tensor("slotp", [128, bpc * (csum - half)], BF16,
                           kind="ExternalInput")
    invd = nc.dram_tensor("invd", [128, bpc], F32, kind="ExternalInput")
    xod = nc.dram_tensor("xod", [npc, D], F32, kind="ExternalInput")
    wtd = nc.dram_tensor("wtd", [D, D], F32, kind="ExternalInput")
    iotad = nc.dram_tensor("iotad", [128, SW], BF16, kind="ExternalInput")
    outd = nc.dram_tensor("out", [npc, D], F32, kind="ExternalOutput")

    qcall = 0

    with tile.TileContext(nc) as tc:
        with tc.tile_pool(name="const", bufs=1) as const, \
             tc.tile_pool(name="ework", bufs=4) as ework, \
             tc.tile_pool(name="sbwork", bufs=4) as sbwork, \
             tc.tile_pool(name="psum1", bufs=2, space="PSUM") as psum1p, \
             tc.tile_pool(name="psum2", bufs=2, space="PSUM") as psum2p:

            wt_t = const.tile([128, D], F32)
            nc.sync.dma_start(out=wt_t[:], in_=wtd[:, :])
            iota_t = const.tile([128, SW], BF16)
            nc.sync.dma_start(out=iota_t[:], in_=iotad[:, :])
            inv_t = const.tile([128, bpc], F32)
            nc.sync.dma_start(out=inv_t[:], in_=invd[:, :])
            slotv_t = const.tile([128, bpc * half], BF16)
            nc.sync.dma_start(out=slotv_t[:], in_=slotv[:, :])
            slotp_t = const.tile([128, bpc * (csum - half)], BF16)
            nc.sync.dma_start(out=slotp_t[:], in_=slotp[:, :])

            base_col = 0
            b0 = 0
            for g_i, G in enumerate(groups):
                nv = G * half
                np_ = G * (csum - half)
                s_v = ework.tile([128, nv, SW], BF16, tag="Sv")
                nc.vector.tensor_tensor(
                    out=s_v[:, :, :],
                    in0=slotv_t[:, b0 * half:b0 * half + nv].unsqueeze(2)
                        .to_broadcast([128, nv, SW]),
                    in1=iota_t[:].unsqueeze(1).to_broadcast([128, nv, SW]),
                    op=mybir.AluOpType.is_equal,
                )
                s_p = ework.tile([128, np_, SW], BF16, tag="Sp")
                nc.vector.tensor_tensor(
                    out=s_p[:, :, :],
                    in0=slotp_t[:, b0 * (csum - half):
                                b0 * (csum - half) + np_].unsqueeze(2)
                        .to_broadcast([128, np_, SW]),
                    in1=iota_t[:].unsqueeze(1).to_broadcast([128, np_, SW]),
                    op=mybir.AluOpType.is_equal,
                )
                eb_list = []
                for q in range(4):
                    nidx = G * c_list[q] * 128
                    ncols = nidx // 16
                    idx_t = ework.tile([128, ncols], I16, tag=f"idx{q}")
                    nc.sync.dma_start(
                        out=idx_t[:],
                        in_=idxd[:, base_col:base_col + ncols])
                    ef = ework.tile([128, G * c_list[q], 128], BF16,
                                    tag=f"ef{q}")
                    c0 = 0
                    while c0 * 128 < nidx:
                        n_call = min(cap, nidx - c0 * 128)
                        nch = n_call // 128
                        nc.gpsimd.dma_gather(
                            out_ap=ef[:, c0:c0 + nch, :],
                            in_ap=xq[q * qrows:, :],
                            idxs_ap=idx_t[:, c0 * 8:(c0 + nch) * 8],
                            num_idxs=n_call,
                            num_idxs_reg=n_call,
                            elem_size=D,
                            queue_num=qcall % 4,
                        )
                        qcall += 1
                        c0 += nch
                    eb_list.append(ef)
                    base_col += ncols

                for bl in range(G):
                    b = b0 + bl
                    p1 = psum1p.tile([128, SW], F32, tag="p1")
                    cglob = 0
                    for q in range(4):
                        cq = c_list[q]
                        for k in range(cq):
                            if cglob < half:
                                s_rhs = s_v[:, bl * half + cglob, :]
                            else:
                                s_rhs = s_p[:, bl * (csum - half)
                                            + (cglob - half), :]
                            nc.tensor.matmul(
                                out=p1[:, :],
                                lhsT=eb_list[q][:, bl * cq + k, :],
                                rhs=s_rhs,
                                start=(cglob == 0),
                                stop=(cglob == csum - 1),
                            )
                            cglob += 1
                    agg_t = sbwork.tile([128, 128], F32, tag="aggT")
                    nc.scalar.copy(agg_t[:], p1[:, :])
                    p2 = psum2p.tile([128, 128], F32, tag="p2")
                    nc.tensor.matmul(out=p2[:, :], lhsT=agg_t[:],
                                     rhs=wt_t[:], start=True, stop=True)
                    xo_t = sbwork.tile([128, 128], F32, tag="xo")
                    nc.scalar.dma_start(out=xo_t[:],
                                        in_=xod[b * 128:(b + 1) * 128, :])
                    hb_t = sbwork.tile([128, 128], F32, tag="hb")
                    nc.scalar.activation(
                        hb_t[:], p2[:, :],
                        mybir.ActivationFunctionType.Relu,
                        scale=inv_t[:, b:b + 1])
                    ob_t = sbwork.tile([128, 128], F32, tag="ob")
                    nc.vector.tensor_add(ob_t[:], hb_t[:], xo_t[:])
                    nc.sync.dma_start(out=outd[b * 128:(b + 1) * 128, :],
                                      in_=ob_t[:])
                b0 += G
    nc.compile()
    _BUILD_CACHE[key] = nc
    return nc


def _pack_blocks(qd, nblocks, cap, node_cap=128, reserve0=8):
    n = qd.shape[0]
    loads = np.zeros((nblocks, 4), dtype=np.int64)
    slots = np.full(nblocks, node_cap, dtype=np.int64)
    assign = np.full(n, -1, dtype=np.int64)
    order = np.lexsort((-qd.sum(1), -qd.max(1)))
    total = max(int(qd.sum()), 1)
    placed_w = 0
    for i in order:
        cap_eff = cap - int(np.ceil(reserve0 * (1.0 - placed_w / total)))
        score = (loads + qd[i]).max(1).astype(np.float64)
        score += (node_cap - slots) * 1e-3
        score[(slots <= 0) | ((loads + qd[i]) > cap_eff).any(1)] = 1e18
        b = int(np.argmin(score))
        if score[b] >= 1e18:
            score2 = (loads + qd[i]).max(1).astype(np.float64)
            score2[(slots <= 0) | ((loads + qd[i]) > cap).any(1)] = 1e18
            b = int(np.argmin(score2))
            if score2[b] >= 1e18:
                return None
        assign[i] = b
        loads[b] += qd[i]
        slots[b] -= 1
        placed_w += int(qd[i].sum())
    return assign


def _preprocess(x, src, dst, W, n_nodes, ncores, gmax=4, cap=1024):
    D_ = x.shape[1]
    min_bpc = -(-n_nodes // (ncores * 128))

    deg = np.bincount(dst, minlength=n_nodes)
    inv = 1.0 / np.maximum(deg, 1).astype(np.float32)
    zero_deg = np.where(deg == 0)[0]
    if len(zero_deg):
        src = np.concatenate([src, zero_deg])
        dst = np.concatenate([dst, zero_deg])

    # quarters split over the REAL node range (npad//4 skews quarter 3
    # small and drives quarters 0-2 to exactly the cap -> unpackable)
    qrows = -(-n_nodes // 4)
    quarter = (src // qrows).astype(np.int64)

    assign = None
    for bpc in range(min_bpc + 2, min_bpc + 8):
        npc = bpc * 128
        npad = npc * ncores
        nblocks = ncores * bpc
        qd = np.zeros((npad, 4), dtype=np.int64)
        np.add.at(qd, (dst, quarter), 1)
        assign = _pack_blocks(qd, nblocks, cap=512)
        if assign is not None:
            c_list = [4, 4, 4, 4]
            break
    if assign is None:
        bpc = min_bpc + 2
        npc = bpc * 128
        npad = npc * ncores
        nblocks = ncores * bpc
        assign = np.arange(npad) // 128
        cmat = np.zeros((nblocks, 4), dtype=np.int64)
        np.add.at(cmat, (assign[dst], quarter), 1)
        c_list = [int(-(-cmat[:, q].max() // 128)) for q in range(4)]
        c_list = [max(c, 1) for c in c_list]
    csum = int(sum(c_list))
    half = csum // 2
    qoff = np.concatenate([[0], np.cumsum(c_list)]).astype(int)

    blk_of = assign[:npad].copy()
    unassigned = np.where(blk_of < 0)[0]
    if len(unassigned):
        counts = np.bincount(blk_of[blk_of >= 0], minlength=nblocks)
        free = []
        for b in range(nblocks):
            free.extend([b] * (128 - counts[b]))
        blk_of[unassigned] = np.array(free[:len(unassigned)], dtype=np.int64)
    order_nodes = np.argsort(blk_of, kind="stable")
    slot_of = np.zeros(npad, dtype=np.int64)
    counts = np.bincount(blk_of, minlength=nblocks)
    assert counts.max() <= 128, "block overflow"
    start = np.concatenate([[0], np.cumsum(counts)])
    slot_of[order_nodes] = np.arange(npad) - start[blk_of[order_nodes]]
    pos_of = blk_of * 128 + slot_of
    perm = np.zeros(npad, dtype=np.int64)
    perm[pos_of] = np.arange(npad)

    groups = _plan_groups(bpc, gmax)
    call_base = np.zeros((len(groups), 4), dtype=np.int64)
    pos = 0
    for gi, G in enumerate(groups):
        for q in range(4):
            call_base[gi, q] = pos
            pos += G * c_list[q] * 128
    totslots = pos
    assert totslots == bpc * csum * 128

    g_of_block = np.zeros(bpc, dtype=np.int64)
    boff_of_block = np.zeros(bpc, dtype=np.int64)
    b = 0
    for gi, G in enumerate(groups):
        for j in range(G):
            g_of_block[b] = gi
            boff_of_block[b] = j
            b += 1

    blk_e = blk_of[dst]
    slot_e = slot_of[dst]
    order = np.lexsort((src, quarter, blk_e))
    src_s = src[order]
    q_s = quarter[order]
    blk_s = blk_e[order]
    slot_s = slot_e[order]

    seg_id = blk_s * 4 + q_s
    seg_counts = np.bincount(seg_id, minlength=nblocks * 4)
    cmat = seg_counts.reshape(nblocks, 4)
    for q in range(4):
        assert cmat[:, q].max() <= c_list[q] * 128, \
            f"quarter {q} overflow: {cmat[:, q].max()}"
    seg_start = np.concatenate([[0], np.cumsum(seg_counts)])
    rank = np.arange(len(src_s)) - seg_start[seg_id]
    core_e = blk_s // bpc
    bl_local = blk_s % bpc
    cq_e = np.array(c_list)[q_s]
    pos_e = (call_base[g_of_block[bl_local], q_s]
             + boff_of_block[bl_local] * cq_e * 128 + rank)

    idx16 = np.zeros((ncores, 16, totslots // 16), dtype=np.int16)
    idx16[core_e, pos_e % 16, pos_e // 16] = (src_s - q_s * qrows).astype(
        np.int16)
    idx_rep = np.tile(idx16, (1, 8, 1))

    slot_arr = np.full((ncores, 128, bpc * csum), PAD_SLOT, dtype=np.float32)
    chunk_in_block = qoff[q_s] + rank // 128
    slot_arr[core_e, rank % 128, bl_local * csum + chunk_in_block] = \
        slot_s.astype(np.float32)
    slot_bf = slot_arr.astype(ml_dtypes.bfloat16)
    s4 = slot_bf.reshape(ncores, 128, bpc, csum)
    slot_v = np.ascontiguousarray(
        s4[:, :, :, :half].reshape(ncores, 128, bpc * half))
    slot_p = np.ascontiguousarray(
        s4[:, :, :, half:].reshape(ncores, 128, bpc * (csum - half)))

    inv_arr = np.ones((ncores, 128, bpc), dtype=np.float32)
    nodes = np.arange(n_nodes)
    inv_arr[blk_of[nodes] // bpc, slot_of[nodes], blk_of[nodes] % bpc] = inv

    xpad = np.zeros((npad, D_), dtype=np.float32)
    xpad[:n_nodes] = x
    xpad_bf = xpad.astype(ml_dtypes.bfloat16)
    xperm = xpad[perm]

    iota = np.tile(np.arange(SW, dtype=np.float32)[None, :],
                   (128, 1)).astype(ml_dtypes.bfloat16)
    wt = np.ascontiguousarray(W.T.astype(np.float32))

    in_maps = []
    for c in range(ncores):
        in_maps.append({
            "xq": xpad_bf,
            "idxd": np.ascontiguousarray(idx_rep[c]),
            "slotv": slot_v[c],
            "slotp": slot_p[c],
            "invd": np.ascontiguousarray(inv_arr[c]),
            "xod": np.ascontiguousarray(xperm[c * npc:(c + 1) * npc]),
            "wtd": wt,
            "iotad": iota,
        })
    return in_maps, c_list, groups, bpc, npad, qrows, perm


def kernel(x, src, dst, W, n_nodes=None, trace=False, cap=None, gmax=None):
    global LAST_RESULTS
    x = np.ascontiguousarray(np.asarray(x, dtype=np.float32))
    W = np.ascontiguousarray(np.asarray(W, dtype=np.float32))
    src = np.asarray(src).astype(np.int64)
    dst = np.asarray(dst).astype(np.int64)
    if n_nodes is None:
        n_nodes = x.shape[0]
    if cap is None:
        cap = GATHER_CAP
    if gmax is None:
        gmax = 4

    in_maps, c_list, groups, bpc, npad, qrows, perm = _preprocess(
        x, src, dst, W, n_nodes, NCORES, gmax=gmax, cap=cap)
    nc = _build(tuple(c_list), tuple(groups), bpc, npad, qrows, cap)
    res = bass_utils.run_bass_kernel_spmd(
        nc, in_maps, core_ids=list(range(NCORES)), trace=trace)
    LAST_RESULTS = res
    npc = bpc * 128
    out_perm = np.concatenate([res.results[c]["out"] for c in range(NCORES)],
                              axis=0)
    out = np.zeros((n_nodes, x.shape[1]), dtype=np.float32)
    valid = perm < n_nodes
    out[perm[valid]] = out_perm[valid]
    return out
